# revision 4
# baseline (speedup 1.0000x reference)
"""CVRP pointer-network decoder, data-parallel over 8 NeuronCores.

Strategy: pure batch data-parallelism (2048 -> 8 x 256) via jax shard_map on
the 8 visible neuron devices. The big algorithmic win vs the reference: the
K / V / Kp projections of encoder_inputs (loop-invariant) are computed ONCE
before the 150-step decode loop instead of inside prob_attention every step
(a ~150x FLOP reduction on the dominant term).
"""
import numpy as np

B, N, D = 2048, 101, 128
H = 8
HD = D // H
ND = 1
NSTEPS = 150
NCORES = 8

_compiled = None


def _build():
    import jax
    import jax.numpy as jnp
    from jax.sharding import Mesh, PartitionSpec as P
    from jax.experimental.shard_map import shard_map
    from functools import partial

    devs = jax.devices()[:NCORES]
    mesh = Mesh(np.asarray(devs), ("b",))

    norm_mha = 1.0 / np.sqrt(D / H)
    norm_prob = 1.0 / np.sqrt(D)

    def decode_shard(enc, pool, cap, demand, fc_w, fc1_w, pk_w, mw_w, mk_w, mv_w, mfc_w):
        b = enc.shape[0]
        n = N
        barange = jnp.arange(b)
        demands = demand.reshape(b, n)
        cap0 = cap[0, 0]

        # ---- loop-invariant projections (the big win) ----
        K = (enc @ mk_w.T).reshape(b, n, H, HD).transpose(0, 2, 1, 3)  # b,h,n,hd
        V = (enc @ mv_w.T).reshape(b, n, H, HD).transpose(0, 2, 1, 3)
        Kp = enc @ pk_w.T                                              # b,n,d

        def update_mask_oh(dyn_cap, index, onehot, mask1):
            # scatter-free: one-hot max for the visited set, concat for depot col
            mask1 = jnp.maximum(mask1, onehot)
            go_depot = (index < ND).astype(mask1.dtype)[:, None]
            mask1 = jnp.concatenate([go_depot, mask1[:, ND:]], axis=1)
            mask = jnp.where(demands > dyn_cap, 1.0, mask1)
            # min over customer cols > 0.5 <=> all masked
            all_done = (jnp.min(mask[:, ND:], axis=1) > 0.5).astype(mask.dtype)[:, None]
            depot_col = jnp.where(all_done > 0.5, 0.0, mask[:, :ND])
            mask = jnp.concatenate([depot_col, mask[:, ND:]], axis=1)
            return mask, mask1

        dyn_cap0 = cap.reshape(b, -1)
        mask1_init = jnp.zeros((b, n), enc.dtype)
        onehot0 = jnp.zeros((b, n), enc.dtype).at[:, 0].set(1.0)
        mask0, mask1_0 = update_mask_oh(
            dyn_cap0, jnp.zeros((b,), jnp.int32), onehot0, mask1_init)
        input0 = enc[:, 0, :]

        def step(carry, _):
            _input, pl, dyn_cap, mask, mask1 = carry
            dec = jnp.concatenate([_input, dyn_cap], axis=-1) @ fc_w.T
            pl = pl @ fc1_w.T
            dec = dec + pl
            # ---- attention with precomputed K/V/Kp ----
            Q = (dec @ mw_w.T).reshape(b, H, HD)
            comp = norm_mha * jnp.einsum('bhd,bhnd->bhn', Q, K)
            comp = jnp.where(mask[:, None, :] > 0.5, jnp.float32(-1e30), comp)
            cm = jnp.max(comp, axis=-1, keepdims=True)
            ce = jnp.exp(comp - cm)
            sc = ce / jnp.sum(ce, axis=-1, keepdims=True)
            out = jnp.einsum('bhn,bhnd->bhd', sc, V).reshape(b, D) @ mfc_w.T
            comp2 = norm_prob * jnp.einsum('bd,bnd->bn', out, Kp)
            x = jnp.tanh(comp2) * 10.0
            logits = jnp.where(mask > 0.5, jnp.float32(-1e30), x)

            # argmax via single-operand reduces (neuronxcc can't lower
            # variadic reduce): first-max index through an iota-min trick.
            m = jnp.max(logits, axis=-1, keepdims=True)
            iota = jnp.arange(n, dtype=logits.dtype)[None, :]
            index_f = jnp.min(jnp.where(logits >= m, iota, jnp.float32(n)), axis=-1)
            index = index_f.astype(jnp.int32)
            onehot = (iota == index_f[:, None]).astype(logits.dtype)

            # log_p at the argmax = max - logsumexp
            lse = m[:, 0] + jnp.log(jnp.sum(jnp.exp(logits - m), axis=-1))
            log_p = m[:, 0] - lse
            is_done = (jnp.sum(mask1[:, 1:], axis=1) >= (n - 1)).astype(log_p.dtype)
            log_p = log_p * (1.0 - is_done)

            cur = jnp.sum(demands * onehot, axis=-1, keepdims=True)
            new_cap = dyn_cap - cur
            new_cap = jnp.where((index < ND)[:, None], cap0, new_cap)
            mask, mask1 = update_mask_oh(new_cap, index, onehot, mask1)
            new_input = jnp.einsum('bn,bnd->bd', onehot, enc)
            return (new_input, pl, new_cap, mask, mask1), (index, log_p)

        carry0 = (input0, pool, dyn_cap0, mask0, mask1_0)
        _, (actions, log_ps) = jax.lax.scan(step, carry0, None, length=NSTEPS)
        return actions.T.astype(jnp.int32), jnp.sum(log_ps, axis=0)

    spec_b = P("b")
    spec_r = P()  # replicated weights
    fn = shard_map(
        decode_shard,
        mesh=mesh,
        in_specs=(spec_b, spec_b, spec_b, spec_b,
                  spec_r, spec_r, spec_r, spec_r, spec_r, spec_r, spec_r),
        out_specs=(spec_b, spec_b),
        check_rep=False,
    )
    return jax.jit(fn)


def kernel(encoder_inputs, pool, capcity, demand, n_steps, num_depots, T,
           fc_w, fc1_w, pk_w, mw_w, mk_w, mv_w, mfc_w):
    global _compiled
    import jax.numpy as jnp

    if _compiled is None:
        _compiled = _build()

    enc = jnp.asarray(np.asarray(encoder_inputs, np.float32))
    pl = jnp.asarray(np.asarray(pool, np.float32))
    cap = jnp.asarray(np.asarray(capcity, np.float32))
    dem = jnp.asarray(np.asarray(demand, np.float32))
    ws = [jnp.asarray(np.asarray(w, np.float32))
          for w in (fc_w, fc1_w, pk_w, mw_w, mk_w, mv_w, mfc_w)]
    actions, log_p = _compiled(enc, pl, cap, dem, *ws)
    return np.asarray(actions, np.int32), np.asarray(log_p, np.float32)
